# revision 48
# baseline (speedup 1.0000x reference)
"""FDLoss kernel for Trainium2 (Bass/Tile), data-parallel over 8 NeuronCores.

Math (a = target.flatten(), b = source.flatten()):
    fdback = where(a<0 & b<0, b-a, a-b)
    loss   = mean((fdback - a)^2)
Per element (case analysis):
    value = (b + relu(-2a) * (b<0))^2
The whole per-element pipeline + free-dim sum runs as ONE custom DVE op per
tile:  body = sq(Src1 + relu(Src0*C0)*(Src1 < Zero)), accum=add
(in0 = a half-tile, in1 = b half-tile, s0 = -2.0), accum_out -> acc[:, tile].

Host-side, each core's shard is quantized to fp8 e4m3 (4x less HBM traffic;
the 51M-element mean keeps the scalar's quantization error ~1e-3) and
repacked so every tile is one contiguous [P, 2*FD] block holding
[a-row | b-row] per partition — one DMA per tile (instead of two),
alternated across the two HWDGE rings (SP and ACT). The start of the
stream is tapered (448..1792-row chunks) so the first DVE op starts ~10us
in; no end taper since the DVE, not the DMA, is the critical path.

Each core writes a [128, N_COLS] partial-sum tile; the host sums the 8 small
tiles in f64 and divides by N (the output is a scalar, so a host-side gather
replaces the all-reduce in the sharding hint).
"""

from operator import add as _operator_add

import ml_dtypes
import numpy as np

import concourse.bacc as bacc
import concourse.mybir as mybir
import concourse.dve_ops as dve_ops
from concourse.dve_ops import DveOp
from concourse.dve_spec import Spec, Src0, Src1, C0, Zero, relu, sq, lower, _has_src1
from concourse.dve_uop import DveOpSpec
from concourse.tile import TileContext
from concourse.bass_utils import run_bass_kernel_spmd

N_CORES = 8
FULL_SHAPE = (64, 256, 56, 56)
TOTAL = 64 * 256 * 56 * 56          # 51,380,224
PER_CORE = TOTAL // N_CORES         # 6,422,528 = 128 * 50,176
P = 128
FD_TOTAL = PER_CORE // P            # 50,176
FD = 3584                           # full-tile rows (per-partition elements)
# 3584-row mid chunks with bufs=6 measured fastest (71.9us; bufs=8 gave
# 72.0-72.7). Larger 7168-row mids amortize per-op overhead better (1.0635
# vs 1.0854 ns/row) but LOSE overall (74.9us): the DMA engines interleave
# descriptors of concurrent instructions on a queue, so a big chunk queued
# behind delays the head chunk's completion semaphore by ~5us (measured),
# stalling the DVE. Shrinking bufs to 4 starves the rings late-stream
# instead (77.6us — chunk k+4's DMA is gated on the DVE freeing buffer k);
# bufs=6 = 3 in flight per ring is the sweet spot.
# The kernel is DVE-bound (fp8 keeps the custom op at 1 elem/lane/cycle), so
# the schedule tapers the START: a medium first chunk gets the first DVE op
# going at ~10us, and the start sizes/ring assignment come from a pipeline
# model calibrated on the measured trace (ring ~0.876 ns/row + ~1.1us
# per-chunk desc-gen boundary, first packet SP@8.7us ACT@9.1us, DVE
# 1.0417 ns/row). No end taper — DMA finishes ~25us before the DVE does.
# Each entry: (rows, ring) with ring 0 = SP, 1 = ACT. Small alternating start
# chunks measured fastest (larger first chunks delay the first DVE op: the
# ring's first-packet latency grows with first-chunk size).
_START = [(448, 0), (448, 1), (448, 0), (896, 1), (1344, 0), (1792, 1), (1792, 0)]
_N_FULL = (FD_TOTAL - sum(n for n, _ in _START)) // FD   # 6 full tiles
assert sum(n for n, _ in _START) + _N_FULL * FD == FD_TOTAL
_SCHED = list(_START) + [(FD, (i + 1) % 2) for i in range(_N_FULL)]
# Chunks streamed as bf16 with a 2x DVE perf-mode program (see _DveOp2x /
# _emit_2x). EMPTY: the 2X_1PORT/2X_2PORT modes DO engage on HW (0.565
# ns/row measured, 2x the REGULAR rate) when the table has 2x slots and the
# instruction sets perf_max, but reusing the REGULAR uop program verbatim in
# those slots produces NaN — the 2x programs need a different (undocumented)
# encoding. Left as a pointer for future work: a correct 2x program enables
# an fp8/bf16 hybrid balancing DMA vs DVE at ~45us (~60us total).
_BF16_CHUNKS = set()
CHUNKS = []
_off = 0
for _i, (_n, _ring) in enumerate(_SCHED):
    CHUNKS.append((_off, _n, _ring, "h" if _i in _BF16_CHUNKS else "v"))
    _off += _n
N_COLS = len(CHUNKS)
BF16_TOTAL = sum(n for _, n, _, c in CHUNKS if c == "h") * 2 * P

_F32 = mybir.dt.float32
# Inputs stream as fp8 e4m3: the loss is a 51M-element mean, so per-element
# quantization error (~0.4% RMS) averages to ~1e-3 relative on the scalar —
# 20x inside the 2e-2 gate — while HBM traffic drops 4x vs f32.
_DT_IN = mybir.dt.float8e4
_NP_IN = ml_dtypes.float8_e4m3
_DT_16 = mybir.dt.bfloat16
_NP_16 = ml_dtypes.bfloat16

_OP_NAME = "FDLOSS_SQ_REDUCE"


class _DveOp2x(DveOp):
    """DveOp whose compiled table also carries a 2X_1PORT program (same uop
    list as REGULAR — the 2x mode double-pumps independent elements through
    the same body; the accum add is order-insensitive)."""

    def compile(self, ver):
        key = ("2x:" + self.name, ver)
        if (r := dve_ops._COMPILE_CACHE.get(key)) is not None:
            return r
        uops = lower(self.spec, ver=ver)
        result = DveOpSpec(
            name=self.name,
            opcode=dve_ops.get_dve_sub_opcode(self.name),
            uops=uops,
            uops_2x=uops,
            uops_2x_2p=uops,
            rd1_en=_has_src1(self.spec),
            perf_max=2,
        )
        got = result.sha(ver)
        if self.uops_sha.get(ver) != got:
            raise ValueError(f"{self.name}: sha drift {ver}: {got}")
        dve_ops._COMPILE_CACHE[key] = result
        return result


def _fdloss_ref(in0, in1, c0, c1, c2):
    """CoreSim reference: (out, accum_out) for the accum-bearing spec."""
    b = np.square(
        in1 + np.maximum(in0 * c0, 0.0) * (in1 < 0.0)
    ).astype(np.float32)
    return b, b.reshape(b.shape[0], -1).sum(axis=-1, keepdims=True)


def _register_op() -> DveOp:
    """Register the fused op in dve_ops' registries (repo is read-only, so we
    extend OPS at runtime — same effect as adding the constant in the file)."""
    for op in dve_ops.OPS:
        if op.name == _OP_NAME:
            return op
    spec = Spec(
        body=sq(Src1 + relu(Src0 * C0) * (Src1 < Zero)),
        accum=_operator_add,
        accum_init=Zero,
        reference=_fdloss_ref,
    )
    row = dve_ops._CUSTOM_DVE_ROW_BASE + len(dve_ops.OPS)
    shas = {}
    for ver in ("v3", "v4"):
        compiled = DveOpSpec(
            name=_OP_NAME,
            opcode=row,
            uops=lower(spec, ver=ver),
            rd1_en=_has_src1(spec),
        )
        shas[ver] = compiled.sha(ver)
    op = DveOp(_OP_NAME, spec, subdim=False, uops_sha=shas)
    dve_ops.OPS.append(op)
    dve_ops._SUB_OPCODE_FOR_NAME[_OP_NAME] = row
    dve_ops.CUSTOM_DVE_SPECS[_OP_NAME] = spec
    return op


_cached_nc = None


def _emit_2x(nc, op, *, out, in0, in1, s0, accum_out):
    """Mirror of BassVectorEngine._custom_dve (TTSS shape, rd1_en, no subdim)
    that emits the instruction with perf_max=1, letting the engine pick the
    2X_1PORT program when the operands are eligible (2-byte, packed, SBUF)."""
    from concourse import bass_isa
    from concourse.dve_ops import get_dve_sub_opcode

    eng = nc.vector
    if op.name not in eng.bass.m.ant_custom_dve_ops:
        eng.bass.m.ant_custom_dve_ops = sorted(
            {*eng.bass.m.ant_custom_dve_ops, op.name}
        )
    shape = bass_isa.CustomDveShape.TTSS
    isa_opcode = eng.bass.isa.Opcode[
        f"NEURON_ISA_TPB_OPCODE_CUSTOM_DVE_ANT_{shape.slot()}"
    ].value
    imm = lambda v: mybir.ImmediateValue(dtype=mybir.dt.float32, value=float(v))
    ins = [
        eng.lower_ap(in0, for_isa=True, opt=True),
        eng.lower_ap(in1, for_isa=True, opt=True),
        imm(s0),
        imm(0.0),
    ]
    outs = [
        eng.lower_ap(out, for_isa=True, opt=True),
        eng.lower_ap(accum_out, for_isa=True),
    ]
    return eng.add_instruction(
        bass_isa.InstCustomDveAnt(
            name=eng.bass.get_next_instruction_name(),
            op_name=op.name,
            rd1_en=True,
            subdim=0,
            imm2=0.0,
            shape=shape,
            row=get_dve_sub_opcode(op.name),
            isa_opcode=isa_opcode,
            ins=ins,
            outs=outs,
            perf_max=2,
        )
    )


def _build_bass():
    """Build the single-core SPMD Bass program (same NEFF on all 8 cores)."""
    fd_op = _register_op()
    nc = bacc.Bacc(trn_type="TRN2")

    # packed layout: per core one flat [2*PER_CORE] tensor; chunk k occupies a
    # contiguous block of P*2*n_k elements laid out as [P, 2, n_k] (per
    # partition: a-row then b-row), so each tile is one linear DMA.
    ab_d = nc.dram_tensor("ab_in", (2 * PER_CORE,), _DT_IN, kind="ExternalInput")
    ab16_d = (
        nc.dram_tensor("ab16_in", (BF16_TOTAL,), _DT_16, kind="ExternalInput")
        if BF16_TOTAL
        else None
    )
    out_d = nc.dram_tensor("partials", (P, N_COLS), _F32, kind="ExternalOutput")

    with TileContext(nc) as tc:
        with (
            tc.tile_pool(name="ab", bufs=5) as ab_pool,
            tc.tile_pool(name="w", bufs=1) as w_pool,
            tc.tile_pool(name="ab16", bufs=3) as ab16_pool,
            tc.tile_pool(name="acc", bufs=1) as acc_pool,
        ):
            acc = acc_pool.tile([P, N_COLS], _F32)
            wt = w_pool.tile([P, FD], _F32)  # write-only scratch for `out`
            wt16 = (
                w_pool.tile([P, FD], _DT_16, tag="w16") if BF16_TOTAL else None
            )
            elem_off = 0
            elem16_off = 0
            for i, (off, n, ring, compute) in enumerate(CHUNKS):
                dma_eng = nc.sync if ring == 0 else nc.scalar
                if compute == "v":
                    abt = ab_pool.tile([P, 2 * FD], _DT_IN, tag="ab")
                    src = ab_d[elem_off : elem_off + P * 2 * n].rearrange(
                        "(p m) -> p m", p=P
                    )
                    elem_off += P * 2 * n
                    dma_eng.dma_start(out=abt[:, : 2 * n], in_=src)
                    nc.vector._custom_dve(
                        fd_op,
                        out=wt[:, :n],
                        in0=abt[:, :n],
                        in1=abt[:, n : 2 * n],
                        s0=-2.0,
                        accum_out=acc[:, i : i + 1],
                    )
                else:
                    abt = ab16_pool.tile([P, 2 * FD], _DT_16, tag="ab16")
                    src = ab16_d[elem16_off : elem16_off + P * 2 * n].rearrange(
                        "(p m) -> p m", p=P
                    )
                    elem16_off += P * 2 * n
                    dma_eng.dma_start(out=abt[:, : 2 * n], in_=src)
                    _emit_2x(
                        nc,
                        fd_op,
                        out=wt16[:, :n],
                        in0=abt[:, :n],
                        in1=abt[:, n : 2 * n],
                        s0=-2.0,
                        accum_out=acc[:, i : i + 1],
                    )
            nc.scalar.dma_start(out=out_d[:], in_=acc[:])

    nc.compile()
    return nc


def _get_nc():
    global _cached_nc
    if _cached_nc is None:
        _cached_nc = _build_bass()
    return _cached_nc


def _pack_inputs(source, target):
    """Repack full inputs into per-core flat streams where chunk k is a
    contiguous [P, 2, n_k] block (a-row then b-row per partition). fp8 ("v")
    chunks go to the ab_in stream, bf16 ("h") chunks to the ab16_in stream."""
    a32 = np.asarray(target, dtype=np.float32).reshape(N_CORES, P, FD_TOTAL)
    b32 = np.asarray(source, dtype=np.float32).reshape(N_CORES, P, FD_TOTAL)
    a8, b8 = a32.astype(_NP_IN), b32.astype(_NP_IN)
    packed = np.zeros((N_CORES, 2 * PER_CORE), dtype=_NP_IN)
    packed16 = np.zeros((N_CORES, max(BF16_TOTAL, 1)), dtype=_NP_16)
    elem_off = 0
    elem16_off = 0
    for off, n, _ring, compute in CHUNKS:
        if compute == "v":
            blk = np.stack(
                [a8[:, :, off : off + n], b8[:, :, off : off + n]], axis=2
            )  # [C, P, 2, n]
            packed[:, elem_off : elem_off + P * 2 * n] = blk.reshape(N_CORES, -1)
            elem_off += P * 2 * n
        else:
            blk = np.stack(
                [
                    a32[:, :, off : off + n].astype(_NP_16),
                    b32[:, :, off : off + n].astype(_NP_16),
                ],
                axis=2,
            )
            packed16[:, elem16_off : elem16_off + P * 2 * n] = blk.reshape(
                N_CORES, -1
            )
            elem16_off += P * 2 * n
    return packed, packed16


def kernel_impl(source, target, trace=False, **run_kwargs):
    """Returns (loss_scalar_f32, BassKernelResults)."""
    packed, packed16 = _pack_inputs(source, target)
    in_maps = [
        {"ab_in": packed[i], **({"ab16_in": packed16[i]} if BF16_TOTAL else {})}
        for i in range(N_CORES)
    ]

    nc = _get_nc()
    res = run_bass_kernel_spmd(
        nc, in_maps, core_ids=list(range(N_CORES)), trace=trace, **run_kwargs
    )
    total = np.float64(0.0)
    for r in res.results:
        total += r["partials"].astype(np.float64).sum()
    loss = np.float32(total / TOTAL)
    return np.array(loss, dtype=np.float32), res


def kernel(**inputs) -> np.ndarray:
    out, _ = kernel_impl(inputs["source"], inputs["target"])
    return out



# revision 49
# speedup vs baseline: 1.1527x; 1.1527x over previous
"""FDLoss kernel for Trainium2 (Bass/Tile), data-parallel over 8 NeuronCores.

Math (a = target.flatten(), b = source.flatten()):
    fdback = where(a<0 & b<0, b-a, a-b)
    loss   = mean((fdback - a)^2)
Per element (case analysis):
    value = (b + relu(-2a) * (b<0))^2
The whole per-element pipeline + free-dim sum runs as ONE custom DVE op per
tile:  body = sq(Src1 + relu(Src0*C0)*(Src1 < Zero)), accum=add
(in0 = a half-tile, in1 = b half-tile, s0 = -2.0), accum_out -> acc[:, tile].

Host-side, each core's shard is quantized to fp8 e4m3 (4x less HBM traffic;
the 51M-element mean keeps the scalar's quantization error ~1e-3) and
repacked so every tile is one contiguous [P, 2*FD] block holding
[a-row | b-row] per partition — one DMA per tile (instead of two),
alternated across the two HWDGE rings (SP and ACT). The start of the
stream is tapered (448..1792-row chunks) so the first DVE op starts ~10us
in; no end taper since the DVE, not the DMA, is the critical path.

Each core writes a [128, N_COLS] partial-sum tile; the host sums the 8 small
tiles in f64 and divides by N (the output is a scalar, so a host-side gather
replaces the all-reduce in the sharding hint).
"""

from operator import add as _operator_add

import ml_dtypes
import numpy as np

import concourse.bacc as bacc
import concourse.mybir as mybir
import concourse.dve_ops as dve_ops
from concourse.dve_ops import DveOp
from concourse.dve_spec import Spec, Src0, Src1, C0, Zero, relu, sq, lower, _has_src1
from concourse.dve_uop import DveOpSpec
from concourse.tile import TileContext
from concourse.bass_utils import run_bass_kernel_spmd

N_CORES = 8
FULL_SHAPE = (64, 256, 56, 56)
TOTAL = 64 * 256 * 56 * 56          # 51,380,224
PER_CORE = TOTAL // N_CORES         # 6,422,528 = 128 * 50,176
P = 128
FD_TOTAL = PER_CORE // P            # 50,176
FD = 3584                           # full-tile rows (per-partition elements)
# 3584-row mid chunks with bufs=6 measured fastest (71.9us; bufs=8 gave
# 72.0-72.7). Larger 7168-row mids amortize per-op overhead better (1.0635
# vs 1.0854 ns/row) but LOSE overall (74.9us): the DMA engines interleave
# descriptors of concurrent instructions on a queue, so a big chunk queued
# behind delays the head chunk's completion semaphore by ~5us (measured),
# stalling the DVE. Shrinking bufs to 4 starves the rings late-stream
# instead (77.6us — chunk k+4's DMA is gated on the DVE freeing buffer k);
# bufs=6 = 3 in flight per ring is the sweet spot.
# The kernel is DVE-bound (fp8 keeps the custom op at 1 elem/lane/cycle), so
# the schedule tapers the START: a medium first chunk gets the first DVE op
# going at ~10us, and the start sizes/ring assignment come from a pipeline
# model calibrated on the measured trace (ring ~0.876 ns/row + ~1.1us
# per-chunk desc-gen boundary, first packet SP@8.7us ACT@9.1us, DVE
# 1.0417 ns/row). No end taper — DMA finishes ~25us before the DVE does.
# Each entry: (rows, ring) with ring 0 = SP, 1 = ACT. Small alternating start
# chunks measured fastest (larger first chunks delay the first DVE op: the
# ring's first-packet latency grows with first-chunk size).
_START = [(448, 0), (448, 1), (896, 0), (896, 1), (896, 0), (1792, 1), (1792, 0)]
_N_FULL = (FD_TOTAL - sum(n for n, _ in _START)) // FD   # 6 full tiles
assert sum(n for n, _ in _START) + _N_FULL * FD == FD_TOTAL
_SCHED = list(_START) + [(FD, (i + 1) % 2) for i in range(_N_FULL)]
# Chunks streamed as bf16 with a 2x DVE perf-mode program (see _DveOp2x /
# _emit_2x). EMPTY: the 2X_1PORT/2X_2PORT modes DO engage on HW (0.565
# ns/row measured, 2x the REGULAR rate) when the table has 2x slots and the
# instruction sets perf_max, but reusing the REGULAR uop program verbatim in
# those slots produces NaN — the 2x programs need a different (undocumented)
# encoding. Left as a pointer for future work: a correct 2x program enables
# an fp8/bf16 hybrid balancing DMA vs DVE at ~45us (~60us total).
_BF16_CHUNKS = set()
CHUNKS = []
_off = 0
for _i, (_n, _ring) in enumerate(_SCHED):
    CHUNKS.append((_off, _n, _ring, "h" if _i in _BF16_CHUNKS else "v"))
    _off += _n
N_COLS = len(CHUNKS)
BF16_TOTAL = sum(n for _, n, _, c in CHUNKS if c == "h") * 2 * P

_F32 = mybir.dt.float32
# Inputs stream as fp8 e4m3: the loss is a 51M-element mean, so per-element
# quantization error (~0.4% RMS) averages to ~1e-3 relative on the scalar —
# 20x inside the 2e-2 gate — while HBM traffic drops 4x vs f32.
_DT_IN = mybir.dt.float8e4
_NP_IN = ml_dtypes.float8_e4m3
_DT_16 = mybir.dt.bfloat16
_NP_16 = ml_dtypes.bfloat16

_OP_NAME = "FDLOSS_SQ_REDUCE"


class _DveOp2x(DveOp):
    """DveOp whose compiled table also carries a 2X_1PORT program (same uop
    list as REGULAR — the 2x mode double-pumps independent elements through
    the same body; the accum add is order-insensitive)."""

    def compile(self, ver):
        key = ("2x:" + self.name, ver)
        if (r := dve_ops._COMPILE_CACHE.get(key)) is not None:
            return r
        uops = lower(self.spec, ver=ver)
        result = DveOpSpec(
            name=self.name,
            opcode=dve_ops.get_dve_sub_opcode(self.name),
            uops=uops,
            uops_2x=uops,
            uops_2x_2p=uops,
            rd1_en=_has_src1(self.spec),
            perf_max=2,
        )
        got = result.sha(ver)
        if self.uops_sha.get(ver) != got:
            raise ValueError(f"{self.name}: sha drift {ver}: {got}")
        dve_ops._COMPILE_CACHE[key] = result
        return result


def _fdloss_ref(in0, in1, c0, c1, c2):
    """CoreSim reference: (out, accum_out) for the accum-bearing spec."""
    b = np.square(
        in1 + np.maximum(in0 * c0, 0.0) * (in1 < 0.0)
    ).astype(np.float32)
    return b, b.reshape(b.shape[0], -1).sum(axis=-1, keepdims=True)


def _register_op() -> DveOp:
    """Register the fused op in dve_ops' registries (repo is read-only, so we
    extend OPS at runtime — same effect as adding the constant in the file)."""
    for op in dve_ops.OPS:
        if op.name == _OP_NAME:
            return op
    spec = Spec(
        body=sq(Src1 + relu(Src0 * C0) * (Src1 < Zero)),
        accum=_operator_add,
        accum_init=Zero,
        reference=_fdloss_ref,
    )
    row = dve_ops._CUSTOM_DVE_ROW_BASE + len(dve_ops.OPS)
    shas = {}
    for ver in ("v3", "v4"):
        compiled = DveOpSpec(
            name=_OP_NAME,
            opcode=row,
            uops=lower(spec, ver=ver),
            rd1_en=_has_src1(spec),
        )
        shas[ver] = compiled.sha(ver)
    op = DveOp(_OP_NAME, spec, subdim=False, uops_sha=shas)
    dve_ops.OPS.append(op)
    dve_ops._SUB_OPCODE_FOR_NAME[_OP_NAME] = row
    dve_ops.CUSTOM_DVE_SPECS[_OP_NAME] = spec
    return op


_cached_nc = None


def _emit_2x(nc, op, *, out, in0, in1, s0, accum_out):
    """Mirror of BassVectorEngine._custom_dve (TTSS shape, rd1_en, no subdim)
    that emits the instruction with perf_max=1, letting the engine pick the
    2X_1PORT program when the operands are eligible (2-byte, packed, SBUF)."""
    from concourse import bass_isa
    from concourse.dve_ops import get_dve_sub_opcode

    eng = nc.vector
    if op.name not in eng.bass.m.ant_custom_dve_ops:
        eng.bass.m.ant_custom_dve_ops = sorted(
            {*eng.bass.m.ant_custom_dve_ops, op.name}
        )
    shape = bass_isa.CustomDveShape.TTSS
    isa_opcode = eng.bass.isa.Opcode[
        f"NEURON_ISA_TPB_OPCODE_CUSTOM_DVE_ANT_{shape.slot()}"
    ].value
    imm = lambda v: mybir.ImmediateValue(dtype=mybir.dt.float32, value=float(v))
    ins = [
        eng.lower_ap(in0, for_isa=True, opt=True),
        eng.lower_ap(in1, for_isa=True, opt=True),
        imm(s0),
        imm(0.0),
    ]
    outs = [
        eng.lower_ap(out, for_isa=True, opt=True),
        eng.lower_ap(accum_out, for_isa=True),
    ]
    return eng.add_instruction(
        bass_isa.InstCustomDveAnt(
            name=eng.bass.get_next_instruction_name(),
            op_name=op.name,
            rd1_en=True,
            subdim=0,
            imm2=0.0,
            shape=shape,
            row=get_dve_sub_opcode(op.name),
            isa_opcode=isa_opcode,
            ins=ins,
            outs=outs,
            perf_max=2,
        )
    )


def _build_bass():
    """Build the single-core SPMD Bass program (same NEFF on all 8 cores)."""
    fd_op = _register_op()
    nc = bacc.Bacc(trn_type="TRN2")

    # packed layout: per core one flat [2*PER_CORE] tensor; chunk k occupies a
    # contiguous block of P*2*n_k elements laid out as [P, 2, n_k] (per
    # partition: a-row then b-row), so each tile is one linear DMA.
    ab_d = nc.dram_tensor("ab_in", (2 * PER_CORE,), _DT_IN, kind="ExternalInput")
    ab16_d = (
        nc.dram_tensor("ab16_in", (BF16_TOTAL,), _DT_16, kind="ExternalInput")
        if BF16_TOTAL
        else None
    )
    out_d = nc.dram_tensor("partials", (P, N_COLS), _F32, kind="ExternalOutput")

    with TileContext(nc) as tc:
        with (
            tc.tile_pool(name="ab", bufs=5) as ab_pool,
            tc.tile_pool(name="w", bufs=1) as w_pool,
            tc.tile_pool(name="ab16", bufs=3) as ab16_pool,
            tc.tile_pool(name="acc", bufs=1) as acc_pool,
        ):
            acc = acc_pool.tile([P, N_COLS], _F32)
            wt = w_pool.tile([P, FD], _F32)  # write-only scratch for `out`
            wt16 = (
                w_pool.tile([P, FD], _DT_16, tag="w16") if BF16_TOTAL else None
            )
            elem_off = 0
            elem16_off = 0
            for i, (off, n, ring, compute) in enumerate(CHUNKS):
                dma_eng = nc.sync if ring == 0 else nc.scalar
                if compute == "v":
                    abt = ab_pool.tile([P, 2 * FD], _DT_IN, tag="ab")
                    src = ab_d[elem_off : elem_off + P * 2 * n].rearrange(
                        "(p m) -> p m", p=P
                    )
                    elem_off += P * 2 * n
                    dma_eng.dma_start(out=abt[:, : 2 * n], in_=src)
                    nc.vector._custom_dve(
                        fd_op,
                        out=wt[:, :n],
                        in0=abt[:, :n],
                        in1=abt[:, n : 2 * n],
                        s0=-2.0,
                        accum_out=acc[:, i : i + 1],
                    )
                else:
                    abt = ab16_pool.tile([P, 2 * FD], _DT_16, tag="ab16")
                    src = ab16_d[elem16_off : elem16_off + P * 2 * n].rearrange(
                        "(p m) -> p m", p=P
                    )
                    elem16_off += P * 2 * n
                    dma_eng.dma_start(out=abt[:, : 2 * n], in_=src)
                    _emit_2x(
                        nc,
                        fd_op,
                        out=wt16[:, :n],
                        in0=abt[:, :n],
                        in1=abt[:, n : 2 * n],
                        s0=-2.0,
                        accum_out=acc[:, i : i + 1],
                    )
            nc.scalar.dma_start(out=out_d[:], in_=acc[:])

    nc.compile()
    return nc


def _get_nc():
    global _cached_nc
    if _cached_nc is None:
        _cached_nc = _build_bass()
    return _cached_nc


def _pack_inputs(source, target):
    """Repack full inputs into per-core flat streams where chunk k is a
    contiguous [P, 2, n_k] block (a-row then b-row per partition). fp8 ("v")
    chunks go to the ab_in stream, bf16 ("h") chunks to the ab16_in stream."""
    a32 = np.asarray(target, dtype=np.float32).reshape(N_CORES, P, FD_TOTAL)
    b32 = np.asarray(source, dtype=np.float32).reshape(N_CORES, P, FD_TOTAL)
    a8, b8 = a32.astype(_NP_IN), b32.astype(_NP_IN)
    packed = np.zeros((N_CORES, 2 * PER_CORE), dtype=_NP_IN)
    packed16 = np.zeros((N_CORES, max(BF16_TOTAL, 1)), dtype=_NP_16)
    elem_off = 0
    elem16_off = 0
    for off, n, _ring, compute in CHUNKS:
        if compute == "v":
            blk = np.stack(
                [a8[:, :, off : off + n], b8[:, :, off : off + n]], axis=2
            )  # [C, P, 2, n]
            packed[:, elem_off : elem_off + P * 2 * n] = blk.reshape(N_CORES, -1)
            elem_off += P * 2 * n
        else:
            blk = np.stack(
                [
                    a32[:, :, off : off + n].astype(_NP_16),
                    b32[:, :, off : off + n].astype(_NP_16),
                ],
                axis=2,
            )
            packed16[:, elem16_off : elem16_off + P * 2 * n] = blk.reshape(
                N_CORES, -1
            )
            elem16_off += P * 2 * n
    return packed, packed16


def kernel_impl(source, target, trace=False, **run_kwargs):
    """Returns (loss_scalar_f32, BassKernelResults)."""
    packed, packed16 = _pack_inputs(source, target)
    in_maps = [
        {"ab_in": packed[i], **({"ab16_in": packed16[i]} if BF16_TOTAL else {})}
        for i in range(N_CORES)
    ]

    nc = _get_nc()
    res = run_bass_kernel_spmd(
        nc, in_maps, core_ids=list(range(N_CORES)), trace=trace, **run_kwargs
    )
    total = np.float64(0.0)
    for r in res.results:
        total += r["partials"].astype(np.float64).sum()
    loss = np.float32(total / TOTAL)
    return np.array(loss, dtype=np.float32), res


def kernel(**inputs) -> np.ndarray:
    out, _ = kernel_impl(inputs["source"], inputs["target"])
    return out

